# revision 21
# baseline (speedup 1.0000x reference)
"""Causal self-attention kernel for Trainium2 (8 NeuronCores).

Problem: B=4, S=2048, D=1024, single-head causal attention, fp32 I/O.
    q = x@Wq+bq; k = x@Wk+bk; v = x@Wv+bv
    out = softmax(mask(q k^T)) v          (no 1/sqrt(d) scaling)

Key algebraic rewrite: scores = (x Wq + bq)(x Wk + bk)^T
  = x (Wq Wk^T) x^T + [row-const] + w[j] + [const]
where row-constant and constant terms drop out of softmax, and
w[j] = x_j . (Wk bq) is a per-key-column bias folded into the mask
data (zero when bq == 0). So with W' = Wk Wq^T precomputed on the
host (weight-only transform), the device computes
    kk^T = W'^T x^T  (one projection instead of Wq+Wk)
    scores[i,j] = x_i . kk_j  (+ mask)
and the Q projection disappears entirely: queries are the raw x^T
tiles already staged in SBUF.

Sharding: 2 cores per batch. Each core runs the SAME program with 8
"slots"; slot j has a static causal budget of K_j = 2j+2 key-tiles
(128 rows each). The host assigns each core 8 query tiles (128 rows)
of its batch such that slot j's tile needs K_j or K_j-1 key tiles:
  variant A tiles [1,2,5,6,9,10,13,14] (needs 2,3,6,7,10,11,14,15)
  variant B tiles [0,3,4,7,8,11,12,15] (needs 1,4,5,8,9,12,13,16)
Both variants sum to 72 key-tile units -> perfectly load balanced; the
per-slot causal mask differences are carried as input data (full-width
additive mask, which also carries the w[j] bias column). Outputs are
scattered back on the host.

A pair-AllGather kk/V split was measured far slower in this
environment (AllGather of 2 MB per rank costs ~113 us wall vs the
~15 us budget that would make dedup pay), so each core redundantly
computes kk^T and V for its whole batch. An SBUF->SBUF DMA xbar
transpose for P^T wedged the device (NRT_EXEC_UNIT_UNRECOVERABLE) —
P^T stays on the PE with an identity matmul.

Numerics: x and W' / Wv in fp16 (1 cycle/row on the PE); PSUM
accumulation fp32; softmax fp32 on ACT/DVE; P/V/AV in fp16.

Engine layout: PE does all matmuls + P-transposes with 512-col
chunks; ACT does kk^T PSUM->SBUF copies, exp, and P^T PSUM->SBUF
copies; DVE does the V bias add (bv folded into V: out =
(P@(V+bv))*linv), the mask add on scores, softmax stats, and output
scaling. Host pre-arranges x/W' in chunk-/dt-major layouts so
streaming DMAs are contiguous; DMA issue order puts w'[dt=0] +
x-chunk0 first with per-dt w' tiles so the first matmul group waits
only on its own slices. Slots run big/small interleaved so softmax
latency hides under score matmuls, ending on the shortest (budget-2)
chain; score/softmax pools (sp, pp) pipeline consecutive slots.
"""
import sys

if '/opt/trn_rl_repo' not in sys.path:
    sys.path.insert(0, '/opt/trn_rl_repo')

import numpy as np

B, S, D = 4, 2048, 1024
P = 128
NT = S // P                      # 16 key/query tiles per batch
SLOTS = [2, 4, 6, 8, 10, 12, 14, 16]   # key-tile budget per slot
MOFF = [0]                              # mask col offset per slot
for _k in SLOTS:
    MOFF.append(MOFF[-1] + _k * P)
MTOT = MOFF[8]                          # 9216
TILES_A = [1, 2, 5, 6, 9, 10, 13, 14]
TILES_B = [0, 3, 4, 7, 8, 11, 12, 15]
NEG = 1e9

_cache = {}


def _build():
    import concourse.bass as bass
    import concourse.mybir as mybir
    import concourse.tile as tile
    from concourse import bacc
    from concourse.masks import make_identity

    F32 = mybir.dt.float32
    F16 = mybir.dt.float16
    BF16 = mybir.dt.bfloat16
    X = mybir.AxisListType.X
    ADD = mybir.AluOpType.add
    Exp = mybir.ActivationFunctionType.Exp

    nc = bacc.Bacc(trn_type="TRN2")
    # inputs (per core) — host pre-arranges chunk-major / dt-major layouts
    # so every streaming DMA reads fully contiguous 1-8KB/partition lines.
    xT16c = nc.dram_tensor("xT16c", [4, P, 4096], F16, kind="ExternalInput")
    xqTc = nc.dram_tensor("xqTc", [2, P, 4096], F16, kind="ExternalInput")
    wmc = nc.dram_tensor("wmc", [8, P, 1024], F16, kind="ExternalInput")
    wv_d = nc.dram_tensor("wv16", [D, D], F16, kind="ExternalInput")
    bv_d = nc.dram_tensor("bv_b", [P, D], F32, kind="ExternalInput")
    mask_d = nc.dram_tensor("mask_t", [P, MTOT], BF16, kind="ExternalInput")
    out_d = nc.dram_tensor("out", [8 * P, D], F32, kind="ExternalOutput")

    wv_r = wv_d.rearrange("(dt p) o -> p dt o", p=P)

    with tile.TileContext(nc) as tc:
        with (tc.tile_pool(name="persist", bufs=1) as persist,
              tc.tile_pool(name="wpool", bufs=2) as wpool,
              tc.tile_pool(name="xstream", bufs=2) as xstream,
              tc.tile_pool(name="qtp", bufs=2) as qtp,
              tc.tile_pool(name="sp", bufs=2) as sp,
              tc.tile_pool(name="pp", bufs=2) as pp,
              tc.tile_pool(name="ptp", bufs=3) as ptp,
              tc.tile_pool(name="op", bufs=2) as op,
              tc.tile_pool(name="stat", bufs=4) as stat,
              tc.tile_pool(name="mm", bufs=4, space="PSUM") as mm,
              tc.tile_pool(name="avp", bufs=2, space="PSUM") as avp,
              tc.tile_pool(name="tps", bufs=2, space="PSUM") as tps):

            # startup-critical order: w'[dt=0], x chunk 0, then the rest of
            # w' (the DMA queue issues serially and the first MM group needs
            # exactly xc0 + w'[dt=0]; per-dt tiles keep the dep that fine).
            wm_sb = []                           # 8 tiles [p, di, c]
            wm_sb.append(persist.tile([P, 8, P], F16, tag="w0", name="wm0"))
            nc.sync.dma_start(wm_sb[0], wmc[0])
            xc0 = xstream.tile([P, 8, 512], F16, tag="x")
            for q in range(4):
                nc.sync.dma_start(xc0[:, 2 * q:2 * q + 2],
                                  xT16c[0].rearrange("p (di w) -> p di w",
                                                     di=8)[:, 2 * q:2 * q + 2])
            for dt in range(1, 8):
                t = persist.tile([P, 8, P], F16, tag=f"w{dt}", name=f"wm{dt}")
                nc.sync.dma_start(t, wmc[dt])
                wm_sb.append(t)

            wv_sb = wpool.tile([P, 8, D], F16, tag="w")
            for h in range(2):
                nc.sync.dma_start(wv_sb[:, :, h * 512:(h + 1) * 512],
                                  wv_r[:, :, h * 512:(h + 1) * 512])

            # --- small constants ---
            bv_sb = persist.tile([P, D], F32, tag="bv")
            nc.sync.dma_start(bv_sb, bv_d[:, :])
            mask_sb = persist.tile([P, MTOT], BF16, tag="mask")
            nc.sync.dma_start(mask_sb, mask_d[:, :])
            ident = persist.tile([P, P], F16, tag="ident")
            make_identity(nc, ident)

            kt = persist.tile([P, 8, S], F16, tag="kt")       # kk^T [do, s]
            v16 = persist.tile([P, NT, D], F16, tag="v16")    # V [s-tile, d]

            # queries: raw x^T tiles (needed only after all projections)
            xq_sb = []
            for g in (1, 0):
                t = qtp.tile([P, 8, 512], F16, tag="xq", name=f"xq{g}")
                nc.sync.dma_start(t, xqTc[g])
                xq_sb.append(t)
            xq_sb = xq_sb[::-1]                  # index by g

            # --- merged kk^T + V projection over shared 512-col x chunks ---
            CW = 512
            for c in range(S // CW):
                if c == 0:
                    xc = xc0
                else:
                    xc = xstream.tile([P, 8, CW], F16, tag="x")
                    nc.sync.dma_start(xc, xT16c[c])
                # kt[:, dt, cols] += W'.T @ x   (stationary = W' column tile)
                for dt in range(8):
                    ps = mm.tile([P, 512], F32, tag="mm", name="psk")
                    for di in range(8):
                        nc.tensor.matmul(ps, wm_sb[dt][:, di],
                                         xc[:, di, :],
                                         start=(di == 0), stop=(di == 7))
                    nc.scalar.copy(kt[:, dt, c * CW:(c + 1) * CW], ps)
                # v16[st, :] = x_st @ Wv + bv   (stationary = x tile;
                # bv folded here so the per-slot output pass skips it:
                # out = (P @ (V+bv)) * linv == (P@V)*linv + bv)
                for st in range(CW // P):
                    for h in range(2):
                        ps = mm.tile([P, 512], F32, tag="mm", name="psv")
                        for di in range(8):
                            nc.tensor.matmul(
                                ps, xc[:, di, st * P:(st + 1) * P],
                                wv_sb[:, di, h * 512:(h + 1) * 512],
                                start=(di == 0), stop=(di == 7))
                        nc.vector.tensor_tensor(
                            out=v16[:, c * (CW // P) + st,
                                    h * 512:(h + 1) * 512],
                            in0=ps, in1=bv_sb[:, h * 512:(h + 1) * 512],
                            op=ADD)

            # --- attention slots, big/small interleaved so every slot's
            # softmax latency is covered by a following slot's score
            # matmuls; ends on the shortest softmax->AV chain (budget 2).
            for j in [7, 3, 6, 2, 5, 4, 1, 0]:
                xq = xq_sb[j // 4]
                lcol = (j % 4) * P
                K = SLOTS[j]
                kcols = K * P

                s_sb = sp.tile([P, S], F32, tag="s")
                for c0 in range(0, kcols, 512):
                    w = min(512, kcols - c0)
                    psf = mm.tile([P, 512], F32, tag="mm", name="pss")
                    ps = psf[:, :w]
                    for di in range(8):
                        nc.tensor.matmul(ps, xq[:, di, lcol:lcol + P],
                                         kt[:, di, c0:c0 + w],
                                         start=(di == 0), stop=(di == 7))
                    nc.vector.tensor_tensor(
                        out=s_sb[:, c0:c0 + w], in0=ps,
                        in1=mask_sb[:, MOFF[j] + c0:MOFF[j] + c0 + w],
                        op=ADD)

                m_val = stat.tile([P, 1], F32, tag="m")
                nc.vector.reduce_max(out=m_val, in_=s_sb[:, :kcols], axis=X)
                negm = stat.tile([P, 1], F32, tag="negm")
                nc.scalar.mul(negm, m_val, -1.0)
                l_val = stat.tile([P, 1], F32, tag="l")
                p16 = pp.tile([P, S], F16, tag="p")
                nc.scalar.activation(out=p16[:, :kcols], in_=s_sb[:, :kcols],
                                     func=Exp, bias=negm, scale=1.0,
                                     accum_out=l_val)
                linv = stat.tile([P, 1], F32, tag="linv")
                nc.vector.reciprocal(linv, l_val)

                av0 = avp.tile([P, 512], F32, tag="av")
                av1 = avp.tile([P, 512], F32, tag="av")
                for ki in range(K):
                    tp = tps.tile([P, P], F16, tag="tp")
                    nc.tensor.transpose(tp, p16[:, ki * P:(ki + 1) * P], ident)
                    pt_sb = ptp.tile([P, P], F16, tag="pt")
                    nc.scalar.copy(pt_sb, tp)
                    nc.tensor.matmul(av0, pt_sb, v16[:, ki, 0:512],
                                     start=(ki == 0), stop=(ki == K - 1))
                    nc.tensor.matmul(av1, pt_sb, v16[:, ki, 512:1024],
                                     start=(ki == 0), stop=(ki == K - 1))
                for h, av in enumerate((av0, av1)):
                    o = op.tile([P, 512], F32, tag="o")
                    nc.vector.tensor_scalar_mul(out=o, in0=av, scalar1=linv)
                    nc.sync.dma_start(
                        out_d[j * P:(j + 1) * P, h * 512:(h + 1) * 512], o)

    nc.compile()
    return nc


def _host_inputs(x, Wq, bq, Wk, bk, Wv, bv):
    import ml_dtypes
    x = np.asarray(x, dtype=np.float32)
    Wq = np.asarray(Wq, dtype=np.float32)
    Wk = np.asarray(Wk, dtype=np.float32)
    Wv = np.asarray(Wv, dtype=np.float32)
    bq = np.asarray(bq, dtype=np.float32)
    bv = np.asarray(bv, dtype=np.float32)

    bv_b = np.ascontiguousarray(np.broadcast_to(bv, (P, D)))
    Wv16 = Wv.astype(np.float16)
    # W' = Wk Wq^T so that the device's kt = W'^T x^T = (Wq Wk^T) x^T
    Wm = (Wk @ Wq.T).astype(np.float16)
    # per-key-column score bias from bq (zero when bq == 0); the q-row
    # bias term (x Wq).bk is constant per row and drops out of softmax
    u2 = Wk @ bq                                   # [D]
    wcol = x.reshape(B * S, D) @ u2                # [B*S]
    wcol = wcol.reshape(B, S)

    def dt_major(W16):
        # [D_in, D_out] -> [dt, p, di*128]: stationary tile (dt, di) is
        # contiguous per partition, per-dt DMA is one 2KB/partition line
        w = W16.reshape(8, P, 8, P)               # (di, p, dt, c)
        return np.ascontiguousarray(w.transpose(2, 1, 0, 3).reshape(8, P, 1024))

    wmc = dt_major(Wm)

    # full-width masks per (variant, batch): [P, MTOT] fp32 -> bf16;
    # carries -NEG on causally-invalid cols and the wcol bias elsewhere.
    pgrid = np.arange(P)[:, None]
    masks = {}
    for b in range(B):
        for var, tiles in ((0, TILES_A), (1, TILES_B)):
            mk = np.empty((P, MTOT), dtype=np.float32)
            for j in range(8):
                t = tiles[j]
                K = SLOTS[j]
                cgrid = np.arange(K * P)[None, :]
                valid = cgrid <= (t * P + pgrid)
                mk[:, MOFF[j]:MOFF[j + 1]] = np.where(
                    valid, wcol[b, :K * P][None, :], -NEG)
            masks[(b, var)] = mk.astype(ml_dtypes.bfloat16)

    def chunk_major(xT, n):
        # [D, W] -> [n, p, di*512]: per-chunk DMA is one fully
        # contiguous 8KB/partition line
        w = xT.shape[1] // n
        a = xT.reshape(8, P, n, w).transpose(2, 1, 0, 3)   # (c, p, di, w)
        return np.ascontiguousarray(a.reshape(n, P, 8 * w))

    xT16_b = [x[b].T.astype(np.float16) for b in range(B)]
    in_maps = []
    for c in range(8):
        b, var = c // 2, c % 2
        tiles = TILES_A if var == 0 else TILES_B
        qcols = np.concatenate(
            [np.arange(t * P, (t + 1) * P) for t in tiles])
        xqT16 = xT16_b[b][:, qcols]
        in_maps.append({
            "xT16c": chunk_major(xT16_b[b], 4),
            "xqTc": chunk_major(xqT16, 2),
            "wmc": wmc, "wv16": Wv16,
            "bv_b": bv_b,
            "mask_t": masks[(b, var)],
        })
    return in_maps


def _assemble(results):
    full = np.empty((B, S, D), dtype=np.float32)
    for c in range(8):
        b, var = c // 2, c % 2
        tiles = TILES_A if var == 0 else TILES_B
        out = results[c]["out"]
        for j, t in enumerate(tiles):
            full[b, t * P:(t + 1) * P, :] = out[j * P:(j + 1) * P, :]
    return full


def kernel(x, Wq, bq, Wk, bk, Wv, bv):
    from concourse.bass_utils import run_bass_kernel_spmd
    if "nc" not in _cache:
        _cache["nc"] = _build()
    nc = _cache["nc"]
    in_maps = _host_inputs(x, Wq, bq, Wk, bk, Wv, bv)
    res = run_bass_kernel_spmd(nc, in_maps, core_ids=list(range(8)))
    return _assemble(res.results)
